# revision 1
# baseline (speedup 1.0000x reference)
"""ANR sparse-attention recommender on 8 Trainium2 NeuronCores.

Strategy (data-parallel on batch, vocab-sharded pre-projection):
  P1: each core projects its 1/8 vocab shard through PROJ_EXT [300,64]
      (cols 0..49 = per-aspect projection (a,h)-major; cols 50..59 = the
      w=0 and w=2 window weights folded in; center w=1 computed on device)
  P2: AllGather the projected table gtab [50176, 64] f32 (12.8MB)
  P3: per-core dma_gather of PAIR rows (gtab viewed [25088, 128], 512B elem,
      idx16 = token_id//2 host-laid-out in the Q7 16-partition wrap) +
      parity select on DVE.  Token slot j -> (partition j%128, col j//128);
      partition p = 4*item + quarter, col t -> l = 125*quarter + t.
  P4: center logit via DVE mult+reduce; window shifts along the free dim
      (+ PE shift-matrix edge fixups); softmax over l via free-reduce +
      selector-matmul cross-quarter sum; rep = attn-weighted reduce +
      selector-matmul; tiny co-attention on DVE with host-expanded weights.
"""
import numpy as np

import concourse.bass as bass
import concourse.bacc as bacc
import concourse.mybir as mybir
import concourse.tile as tile
from concourse.bass_utils import run_bass_kernel_spmd

A, L, D, H1, H2, CWS = 5, 500, 300, 10, 50, 3
V, NU, NI, B = 50000, 20000, 20000, 256
NCORE, BLOC = 8, 32
SHARD = 6272                 # per-core vocab rows (padded); 8*6272 = 50176
VPAD = SHARD * NCORE
GCOL = 64                    # gtab row: 50 adoc + 5 g0 + 5 g2 + 4 pad
NT = SHARD // 128            # 49 tiles per shard
NTOK = 16000                 # tokens per side per core (32 items x 500)
F32 = mybir.dt.float32
I32 = mybir.dt.int32
I16 = mybir.dt.int16
I8 = mybir.dt.uint8
DCH = [(0, 128), (128, 128), (256, 44)]   # D=300 chunks
MUL = mybir.AluOpType.mult
ADD = mybir.AluOpType.add


def _build_nc():
    nc = bacc.Bacc()
    P = nc.declare_dram_parameter

    u_ids = P("u_ids", [BLOC, 1], I32, isOutput=False)
    i_ids = P("i_ids", [BLOC, 1], I32, isOutput=False)
    u_idx16 = P("u_idx16", [128, 1000], I16, isOutput=False)
    i_idx16 = P("i_idx16", [128, 1000], I16, isOutput=False)
    u_par = P("u_par", [128, 125], I8, isOutput=False)
    i_par = P("i_par", [128, 125], I8, isOutput=False)
    my_shard = P("my_shard", [SHARD, D], F32, isOutput=False)
    pext = P("pext", [D, GCOL], F32, isOutput=False)
    ident = P("ident", [128, 128], F32, isOutput=False)
    p4sel = P("p4sel", [128, BLOC], F32, isOutput=False)
    p4selT = P("p4selT", [BLOC, 128], F32, isOutput=False)
    shdn = P("shdn", [128, 128], F32, isOutput=False)   # out[m]=in[m-1] if m%4!=0
    shup = P("shup", [128, 128], F32, isOutput=False)   # out[m]=in[m+1] if m%4!=3
    e1c = P("e1c", [128, 50], F32, isOutput=False)      # E[a, 10+h] all partitions
    m_exp = P("m_exp", [BLOC, 100], F32, isOutput=False)
    upT_exp = P("upT_exp", [BLOC, 500], F32, isOutput=False)
    ipT_exp = P("ipT_exp", [BLOC, 500], F32, isOutput=False)
    uw_exp = P("uw_exp", [BLOC, 50], F32, isOutput=False)
    iw_exp = P("iw_exp", [BLOC, 50], F32, isOutput=False)
    bu = P("bu", [NU, 1], F32, isOutput=False)
    bi = P("bi", [NI, 1], F32, isOutput=False)
    bg = P("bg", [BLOC, 1], F32, isOutput=False)
    out_ext = P("out", [BLOC, 1], F32, isOutput=True)

    with tile.TileContext(nc) as tc:
        with (
            tc.tile_pool(name="dram", bufs=1, space="DRAM") as DP,
            tc.tile_pool(name="consts", bufs=1) as CP,
            tc.tile_pool(name="p1", bufs=4) as P1,
            tc.tile_pool(name="ps", bufs=1, space="PSUM") as PS,
            tc.tile_pool(name="big", bufs=1) as BG,
            tc.tile_pool(name="work", bufs=2) as WK,
            tc.tile_pool(name="scr", bufs=4) as SC,
        ):
            pv_shard = DP.tile([SHARD, GCOL], F32)
            gtab = DP.tile([VPAD, GCOL], F32, addr_space="Shared")

            # ---- load constants ----
            ident_sb = CP.tile([128, 128], F32)
            nc.sync.dma_start(out=ident_sb[:], in_=ident[:])
            p4sel_sb = CP.tile([128, BLOC], F32)
            nc.sync.dma_start(out=p4sel_sb[:], in_=p4sel[:])
            p4selT_sb = CP.tile([BLOC, 128], F32)
            nc.sync.dma_start(out=p4selT_sb[:], in_=p4selT[:])
            shdn_sb = CP.tile([128, 128], F32)
            nc.sync.dma_start(out=shdn_sb[:], in_=shdn[:])
            shup_sb = CP.tile([128, 128], F32)
            nc.sync.dma_start(out=shup_sb[:], in_=shup[:])
            e1c_sb = CP.tile([128, 50], F32)
            nc.sync.dma_start(out=e1c_sb[:], in_=e1c[:])
            pext_sb = []
            for c, (d0, dn) in enumerate(DCH):
                t = CP.tile([128, GCOL], F32, name=f"pext{c}")
                nc.sync.dma_start(out=t[:dn, :], in_=pext[d0:d0 + dn, :])
                pext_sb.append(t)

            # ---- P1: project vocab shard (4 tiles per load) ----
            for g in range((NT + 3) // 4):
                tlo = g * 4
                thi = min(tlo + 4, NT)
                ng = thi - tlo
                emb = P1.tile([128, 4 * D], F32, tag="emb", bufs=2)
                nc.sync.dma_start(
                    out=emb[:, 0:ng * D],
                    in_=my_shard[tlo * 128:thi * 128, :]
                        .rearrange("(c p) d -> p c d", p=128))
                for t in range(tlo, thi):
                    cc = (t - tlo) * D
                    embT = []
                    for c, (d0, dn) in enumerate(DCH):
                        tp = PS.tile([128, 128], F32, tag="tp", bufs=3)
                        nc.tensor.transpose(out=tp[:dn, :],
                                            in_=emb[:, cc + d0:cc + d0 + dn],
                                            identity=ident_sb[:])
                        eT = P1.tile([128, 128], F32, tag=f"embT{c}")
                        nc.vector.tensor_copy(out=eT[:dn, :], in_=tp[:dn, :])
                        embT.append(eT)
                    pvo = PS.tile([128, GCOL], F32, tag="pvo", bufs=3)
                    for c, (d0, dn) in enumerate(DCH):
                        nc.tensor.matmul(out=pvo[:], lhsT=embT[c][:dn, :],
                                         rhs=pext_sb[c][:dn, :],
                                         start=(c == 0), stop=(c == 2))
                    pvs = P1.tile([128, GCOL], F32, tag="pvs")
                    nc.scalar.copy(out=pvs[:], in_=pvo[:])
                    nc.sync.dma_start(out=pv_shard[t * 128:(t + 1) * 128, :],
                                      in_=pvs[:])

            # ---- P2: AllGather ----
            nc.gpsimd.collective_compute(
                "AllGather", mybir.AluOpType.bypass,
                replica_groups=[list(range(NCORE))],
                ins=[pv_shard[:].opt()], outs=[gtab[:].opt()],
            )
            gtab_pairs_u16 = gtab[:].bitcast(mybir.dt.uint16) \
                                    .rearrange("(v two) e -> v (two e)", two=2)

            # ---- P3+P4 per side ----
            reps = {}
            for side, (idx_p, par_p) in {"u": (u_idx16, u_par),
                                         "i": (i_idx16, i_par)}.items():
                idx_sb = WK.tile([128, 1000], I16, tag="idx")
                nc.sync.dma_start(out=idx_sb[:], in_=idx_p[:])
                par_sb = WK.tile([128, 125], I8, tag="par")
                nc.sync.dma_start(out=par_sb[:], in_=par_p[:])

                # gather in 16-t-block chunks (2048 idxs, single_packet)
                sel = BG.tile([128, 125 * GCOL], F32, tag="sel", bufs=2)
                sel3 = sel[:].rearrange("p (t e) -> p t e", e=GCOL)
                bounds = [(0, 32), (32, 63), (63, 95), (95, 125)]
                for t0, t1 in bounds:
                    nt = (t1 - t0) * 128
                    gr = BG.tile([128, 32 * 128], F32, tag="gr", bufs=2)
                    gr3 = gr[:].rearrange("p (t e) -> p t e", e=128)
                    gr3u = gr[:].bitcast(mybir.dt.uint16) \
                                .rearrange("p (t e) -> p t e", e=256)
                    nc.gpsimd.dma_gather(
                        out_ap=gr3u[:, 0:t1 - t0, :], in_ap=gtab_pairs_u16,
                        idxs_ap=idx_sb[:, t0 * 8:t1 * 8],
                        num_idxs=nt, num_idxs_reg=nt, elem_size=256,
                        single_packet=False)
                    # parity select: sel[p,t,:] = gr[p,t, parity*64 : +64]
                    nc.scalar.copy(out=sel3[:, t0:t1, :],
                                   in_=gr3[:, 0:t1 - t0, 0:GCOL])
                    mask3 = par_sb[:, t0:t1].unsqueeze(2) \
                                            .to_broadcast([128, t1 - t0, GCOL])
                    nc.vector.copy_predicated(out=sel3[:, t0:t1, :], mask=mask3,
                                              data=gr3[:, 0:t1 - t0, GCOL:2 * GCOL])
                adoc = sel3[:, :, 0:50].rearrange("p t (a h) -> p t a h", a=A)
                g0s = sel3[:, :, 50:55]     # [p, t, a]
                g2s = sel3[:, :, 55:60]

                # center logit lgc[p,t,a] = sum_h adoc * E1
                wct = BG.tile([128, 6250], F32, tag="wad", bufs=1)
                wct4 = wct[:].rearrange("p (t a h) -> p t a h", a=A, h=H1)
                e1b = e1c_sb[:].rearrange("p (a h) -> p a h", a=A) \
                               .unsqueeze(1).to_broadcast([128, 125, A, H1])
                nc.vector.tensor_tensor(out=wct4, in0=adoc, in1=e1b, op=MUL)
                lg = WK.tile([128, 625], F32, tag="lg")     # [p, t, a]
                lg3 = lg[:].rearrange("p (t a) -> p t a", a=A)
                nc.vector.tensor_reduce(out=lg3, in_=wct4,
                                        axis=mybir.AxisListType.X,
                                        op=mybir.AluOpType.add)
                # window shifts along t
                nc.vector.tensor_tensor(out=lg3[:, 1:125, :], in0=lg3[:, 1:125, :],
                                        in1=g0s[:, 0:124, :], op=ADD)
                nc.vector.tensor_tensor(out=lg3[:, 0:124, :], in0=lg3[:, 0:124, :],
                                        in1=g2s[:, 1:125, :], op=ADD)
                # cross-quarter edges via PE shift matrices
                e0 = PS.tile([128, A], F32, tag="sps", bufs=2)
                nc.tensor.matmul(out=e0[:], lhsT=shdn_sb[:], rhs=g0s[:, 124, :],
                                 start=True, stop=True)
                nc.vector.tensor_tensor(out=lg3[:, 0, :], in0=lg3[:, 0, :],
                                        in1=e0[:], op=ADD)
                e1m = PS.tile([128, A], F32, tag="sps", bufs=2)
                nc.tensor.matmul(out=e1m[:], lhsT=shup_sb[:], rhs=g2s[:, 0, :],
                                 start=True, stop=True)
                nc.vector.tensor_tensor(out=lg3[:, 124, :], in0=lg3[:, 124, :],
                                        in1=e1m[:], op=ADD)

                # softmax over l (no max shift; logits are tiny)
                E = WK.tile([128, 625], F32, tag="E")
                nc.scalar.activation(out=E[:], in_=lg[:],
                                     func=mybir.ActivationFunctionType.Exp)
                E3 = E[:].rearrange("p (t a) -> p t a", a=A)
                Eat = E[:].rearrange("p (t a) -> p a t", a=A)
                S = SC.tile([128, A], F32, tag="S")
                nc.vector.tensor_reduce(out=S[:], in_=Eat,
                                        axis=mybir.AxisListType.X,
                                        op=mybir.AluOpType.add)
                sit = PS.tile([BLOC, A], F32, tag="sps", bufs=2)
                nc.tensor.matmul(out=sit[:], lhsT=p4sel_sb[:], rhs=S[:],
                                 start=True, stop=True)
                srec = SC.tile([BLOC, A], F32, tag="srec")
                nc.vector.reciprocal(out=srec[:], in_=sit[:])
                sbc = PS.tile([128, A], F32, tag="sps", bufs=2)
                nc.tensor.matmul(out=sbc[:], lhsT=p4selT_sb[:], rhs=srec[:],
                                 start=True, stop=True)
                attn = WK.tile([128, 625], F32, tag="attn")
                attn3 = attn[:].rearrange("p (t a) -> p t a", a=A)
                sbc3 = sbc[:].unsqueeze(1).to_broadcast([128, 125, A])
                nc.vector.tensor_tensor(out=attn3, in0=E3, in1=sbc3, op=MUL)

                # rep: weighted sum of adoc over l, then cross-quarter sum
                wad = BG.tile([128, 6250], F32, tag="wad", bufs=1)
                wad4 = wad[:].rearrange("p (t a h) -> p t a h", a=A, h=H1)
                attnb = attn3.unsqueeze(3).to_broadcast([128, 125, A, H1])
                nc.vector.tensor_tensor(out=wad4, in0=adoc, in1=attnb, op=MUL)
                wsum = WK.tile([128, 50], F32, tag="wsum")
                wad_aht = wad[:].rearrange("p (t ah) -> p ah t", ah=50)
                nc.vector.tensor_reduce(out=wsum[:], in_=wad_aht,
                                        axis=mybir.AxisListType.X,
                                        op=mybir.AluOpType.add)
                repp = PS.tile([BLOC, 50], F32, tag="sps", bufs=2)
                nc.tensor.matmul(out=repp[:], lhsT=p4sel_sb[:], rhs=wsum[:],
                                 start=True, stop=True)
                rep = WK.tile([BLOC, 50], F32, tag=f"rep_{side}")
                nc.vector.tensor_copy(out=rep[:], in_=repp[:])
                reps[side] = rep

            # ---- co-attention (all [32, *] DVE ops) ----
            mexp_sb = CP.tile([BLOC, 100], F32)
            nc.sync.dma_start(out=mexp_sb[:], in_=m_exp[:])
            up_sb = CP.tile([BLOC, 500], F32)
            nc.sync.dma_start(out=up_sb[:], in_=upT_exp[:])
            ip_sb = CP.tile([BLOC, 500], F32)
            nc.sync.dma_start(out=ip_sb[:], in_=ipT_exp[:])
            uw_sb = CP.tile([BLOC, 50], F32)
            nc.sync.dma_start(out=uw_sb[:], in_=uw_exp[:])
            iw_sb = CP.tile([BLOC, 50], F32)
            nc.sync.dma_start(out=iw_sb[:], in_=iw_exp[:])

            ru, ri = reps["u"][:], reps["i"][:]
            ru3 = ru.rearrange("p (a h) -> p a h", a=A)     # [32, 5, 10]
            ri3 = ri.rearrange("p (c k) -> p c k", c=A)
            mexp3 = mexp_sb[:].rearrange("p (h k) -> p h k", h=H1)

            # UdM[b,(a,k)] = sum_h Ud[b,(a,h)] * M[h,k]
            UdM = WK.tile([BLOC, 50], F32, tag="UdM")
            UdM3 = UdM[:].rearrange("p (a k) -> p a k", a=A)
            s50 = SC.tile([BLOC, 50], F32, tag="s50")
            s50_3 = s50[:].rearrange("p (a k) -> p a k", a=A)
            for h in range(H1):
                in0 = ru3[:, :, h].unsqueeze(2).to_broadcast([BLOC, A, H1])
                in1 = mexp3[:, h, :].unsqueeze(1).to_broadcast([BLOC, A, H1])
                nc.vector.tensor_tensor(out=(UdM3 if h == 0 else s50_3),
                                        in0=in0, in1=in1, op=MUL)
                if h > 0:
                    nc.vector.tensor_tensor(out=UdM[:], in0=UdM[:], in1=s50[:], op=ADD)
            # aff[b,(a,c)] = relu(sum_k UdM[b,(a,k)] * Id[b,(c,k)])
            aff0 = WK.tile([BLOC, 25], F32, tag="aff0")
            aff0_3 = aff0[:].rearrange("p (a c) -> p a c", a=A)
            s25 = SC.tile([BLOC, 25], F32, tag="s25")
            s25_3 = s25[:].rearrange("p (a c) -> p a c", a=A)
            for k in range(H1):
                in0 = UdM3[:, :, k].unsqueeze(2).to_broadcast([BLOC, A, A])
                in1 = ri3[:, :, k].unsqueeze(1).to_broadcast([BLOC, A, A])
                nc.vector.tensor_tensor(out=(aff0_3 if k == 0 else s25_3),
                                        in0=in0, in1=in1, op=MUL)
                if k > 0:
                    nc.vector.tensor_tensor(out=aff0[:], in0=aff0[:], in1=s25[:], op=ADD)
            aff = WK.tile([BLOC, 25], F32, tag="aff")
            nc.vector.tensor_scalar_max(out=aff[:], in0=aff0[:], scalar1=0.0)
            aff3 = aff[:].rearrange("p (a c) -> p a c", a=A)

            # Hu1[b,(e,a)] = sum_h up[e,h] Ud[b,(a,h)];  Hi1 likewise
            def proj_h(dst, w_sb, r3):
                dst3 = dst[:].rearrange("p (e a) -> p e a", e=H2)
                s250 = SC.tile([BLOC, 250], F32, tag="s250")
                s250_3 = s250[:].rearrange("p (e a) -> p e a", e=H2)
                w3 = w_sb[:].rearrange("p (h e) -> p h e", h=H1)
                for h in range(H1):
                    in0 = r3[:, :, h].unsqueeze(1).to_broadcast([BLOC, H2, A])
                    in1 = w3[:, h, :].unsqueeze(2).to_broadcast([BLOC, H2, A])
                    nc.vector.tensor_tensor(out=(dst3 if h == 0 else s250_3),
                                            in0=in0, in1=in1, op=MUL)
                    if h > 0:
                        nc.vector.tensor_tensor(out=dst[:], in0=dst[:],
                                                in1=s250[:], op=ADD)

            Hu1 = WK.tile([BLOC, 250], F32, tag="Hu1")
            proj_h(Hu1, up_sb, ru3)
            Hi1 = WK.tile([BLOC, 250], F32, tag="Hi1")
            proj_h(Hi1, ip_sb, ri3)

            # Hu = relu(Hu1 + sum_c Hi1[b,(e,c)] aff[b,(a,c)])
            # Hi = relu(Hi1 + sum_a Hu1[b,(e,a)] aff[b,(a,c)])
            def coatt(dst, h1_self, h1_other, sum_over_c):
                acc = WK.tile([BLOC, 250], F32, tag=f"acc_{sum_over_c}")
                h1o3 = h1_other[:].rearrange("p (e x) -> p e x", e=H2)
                s250b = SC.tile([BLOC, 250], F32, tag="s250b")
                for c in range(A):
                    in0 = h1o3[:, :, c].unsqueeze(2).to_broadcast([BLOC, H2, A])
                    if sum_over_c:   # out index a; aff[:, a, c]
                        in1 = aff3[:, :, c].unsqueeze(1).to_broadcast([BLOC, H2, A])
                    else:            # out index c'; aff[:, c(=a), c']
                        in1 = aff3[:, c, :].unsqueeze(1).to_broadcast([BLOC, H2, A])
                    nc.vector.tensor_tensor(
                        out=s250b[:].rearrange("p (e a) -> p e a", e=H2),
                        in0=in0, in1=in1, op=MUL)
                    src = h1_self[:] if c == 0 else acc[:]
                    nc.vector.tensor_tensor(out=acc[:], in0=src, in1=s250b[:], op=ADD)
                nc.vector.tensor_scalar_max(out=dst[:], in0=acc[:], scalar1=0.0)

            Hu = WK.tile([BLOC, 250], F32, tag="Hu")
            coatt(Hu, Hu1, Hi1, sum_over_c=True)
            Hi = WK.tile([BLOC, 250], F32, tag="Hi")
            coatt(Hi, Hi1, Hu1, sum_over_c=False)

            # imp logits lu[b,a] = sum_e uw[e] Hu[b,(e,a)]
            def imp(dst5, Hx, wx_sb):
                s250c = SC.tile([BLOC, 250], F32, tag="s250c")
                nc.vector.tensor_tensor(
                    out=s250c[:].rearrange("p (e a) -> p e a", e=H2),
                    in0=Hx[:].rearrange("p (e a) -> p e a", e=H2),
                    in1=wx_sb[:].unsqueeze(2).to_broadcast([BLOC, H2, A]), op=MUL)
                v = s250c[:].rearrange("p (e a) -> p a e", e=H2)
                nc.vector.tensor_reduce(out=dst5, in_=v, axis=mybir.AxisListType.X,
                                        op=mybir.AluOpType.add)

            lu = SC.tile([BLOC, A], F32, tag="lu")
            imp(lu[:], Hu, uw_sb)
            li = SC.tile([BLOC, A], F32, tag="li")
            imp(li[:], Hi, iw_sb)
            eu = SC.tile([BLOC, A], F32, tag="eu")
            nc.scalar.activation(out=eu[:], in_=lu[:],
                                 func=mybir.ActivationFunctionType.Exp)
            ei = SC.tile([BLOC, A], F32, tag="ei")
            nc.scalar.activation(out=ei[:], in_=li[:],
                                 func=mybir.ActivationFunctionType.Exp)
            su = SC.tile([BLOC, 1], F32, tag="su")
            nc.vector.tensor_reduce(out=su[:], in_=eu[:], axis=mybir.AxisListType.X,
                                    op=mybir.AluOpType.add)
            si = SC.tile([BLOC, 1], F32, tag="si")
            nc.vector.tensor_reduce(out=si[:], in_=ei[:], axis=mybir.AxisListType.X,
                                    op=mybir.AluOpType.add)
            sur = SC.tile([BLOC, 1], F32, tag="sur")
            nc.vector.reciprocal(out=sur[:], in_=su[:])
            sir = SC.tile([BLOC, 1], F32, tag="sir")
            nc.vector.reciprocal(out=sir[:], in_=si[:])

            # ar[b,a] = sum_h Ud*Id
            arm = SC.tile([BLOC, 50], F32, tag="arm")
            nc.vector.tensor_tensor(out=arm[:], in0=ru, in1=ri, op=MUL)
            ar5 = SC.tile([BLOC, A], F32, tag="ar5")
            nc.vector.tensor_reduce(out=ar5[:],
                                    in_=arm[:].rearrange("p (a h) -> p a h", a=A),
                                    axis=mybir.AxisListType.X, op=mybir.AluOpType.add)
            # R = sum_a eu*ei*ar / (su*si) + biases
            pr = SC.tile([BLOC, A], F32, tag="pr")
            nc.vector.tensor_tensor(out=pr[:], in0=eu[:], in1=ei[:], op=MUL)
            nc.vector.tensor_tensor(out=pr[:], in0=pr[:], in1=ar5[:], op=MUL)
            r0 = SC.tile([BLOC, 1], F32, tag="r0")
            nc.vector.tensor_reduce(out=r0[:], in_=pr[:], axis=mybir.AxisListType.X,
                                    op=mybir.AluOpType.add)
            nc.vector.tensor_tensor(out=r0[:], in0=r0[:], in1=sur[:], op=MUL)
            nc.vector.tensor_tensor(out=r0[:], in0=r0[:], in1=sir[:], op=MUL)

            uid_sb = SC.tile([BLOC, 1], I32, tag="uid")
            nc.sync.dma_start(out=uid_sb[:], in_=u_ids[:])
            iid_sb = SC.tile([BLOC, 1], I32, tag="iid")
            nc.sync.dma_start(out=iid_sb[:], in_=i_ids[:])
            bu_g = SC.tile([BLOC, 1], F32, tag="bu_g")
            nc.gpsimd.indirect_dma_start(
                out=bu_g[:], out_offset=None, in_=bu[:],
                in_offset=bass.IndirectOffsetOnAxis(ap=uid_sb[:, :1], axis=0))
            bi_g = SC.tile([BLOC, 1], F32, tag="bi_g")
            nc.gpsimd.indirect_dma_start(
                out=bi_g[:], out_offset=None, in_=bi[:],
                in_offset=bass.IndirectOffsetOnAxis(ap=iid_sb[:, :1], axis=0))
            bg_sb = SC.tile([BLOC, 1], F32, tag="bg_sb")
            nc.sync.dma_start(out=bg_sb[:], in_=bg[:])
            nc.vector.tensor_tensor(out=r0[:], in0=r0[:], in1=bu_g[:], op=ADD)
            nc.vector.tensor_tensor(out=r0[:], in0=r0[:], in1=bi_g[:], op=ADD)
            nc.vector.tensor_tensor(out=r0[:], in0=r0[:], in1=bg_sb[:], op=ADD)
            nc.sync.dma_start(out=out_ext[:], in_=r0[:])

    nc.finalize()
    return nc


_NC_CACHE = {}
_LAST_IN_MAPS = None


def _idx_layout(ids, docs):
    """idx16 [128,1000] int16 (pair idx) + parity [128,125] f32 for one side."""
    j = np.arange(NTOK)
    p = j % 128
    t = j // 128
    item = p // 4
    l = 125 * (p % 4) + t
    tok = docs[ids[item], l].astype(np.int64)          # [NTOK]
    blk = np.zeros((16, 1000), np.int16)
    blk[j % 16, j // 16] = (tok // 2).astype(np.int16)
    idx16 = np.tile(blk, (8, 1))       # replicated across the 8 Q7 cores
    par = np.zeros((128, 125), np.uint8)
    par[p, t] = (tok % 2).astype(np.uint8)
    return idx16, par


def kernel(U_ids, I_ids, U_docs, I_docs, words_emb, aspect_emb, aspect_proj,
           M, user_proj, user_w, item_proj, item_w, Bu, Bi, Bg):
    U_ids = np.asarray(U_ids).astype(np.int64).reshape(B)
    I_ids = np.asarray(I_ids).astype(np.int64).reshape(B)
    U_docs = np.asarray(U_docs).astype(np.int64)
    I_docs = np.asarray(I_docs).astype(np.int64)
    words_emb = np.asarray(words_emb, np.float32)
    aspect_emb = np.asarray(aspect_emb, np.float32)
    aspect_proj = np.asarray(aspect_proj, np.float32)
    M = np.asarray(M, np.float32)
    user_proj = np.asarray(user_proj, np.float32)
    user_w = np.asarray(user_w, np.float32)
    item_proj = np.asarray(item_proj, np.float32)
    item_w = np.asarray(item_w, np.float32)
    Bu = np.asarray(Bu, np.float32); Bi = np.asarray(Bi, np.float32)
    Bg = np.float32(np.asarray(Bg))

    # ---- host-side parameter prep ----
    pext = np.zeros((D, GCOL), np.float32)
    for a in range(A):
        pext[:, a * 10:(a + 1) * 10] = aspect_proj[a]
    for a in range(A):
        pext[:, 50 + a] = aspect_proj[a] @ aspect_emb[a, 0:10]        # g0 (w=0)
        pext[:, 55 + a] = aspect_proj[a] @ aspect_emb[a, 20:30]       # g2 (w=2)

    words_pad = np.zeros((VPAD, D), np.float32)
    words_pad[:V] = words_emb

    pr = np.arange(128)
    e1 = np.empty((128, 50), np.float32)
    for a in range(A):
        e1[:, a * 10:(a + 1) * 10] = aspect_emb[a, 10:20][None, :]
    consts = {
        "ident": np.eye(128, dtype=np.float32),
        "p4sel": (pr[:, None] // 4 == np.arange(BLOC)[None, :]).astype(np.float32),
        "p4selT": (pr[None, :] // 4 == np.arange(BLOC)[:, None]).astype(np.float32),
        "shdn": ((pr[None, :] == pr[:, None] + 1) &
                 (pr[None, :] % 4 != 0)).astype(np.float32),
        "shup": ((pr[None, :] == pr[:, None] - 1) &
                 (pr[None, :] % 4 != 3)).astype(np.float32),
        "e1c": e1,
        "pext": pext,
        "bu": Bu[:, None].copy(), "bi": Bi[:, None].copy(),
        "bg": np.full((BLOC, 1), Bg, np.float32),
    }
    consts["m_exp"] = np.tile(M.reshape(1, 100), (BLOC, 1)).astype(np.float32)
    consts["upT_exp"] = np.tile(user_proj.T.reshape(1, 500), (BLOC, 1)).astype(np.float32)
    consts["ipT_exp"] = np.tile(item_proj.T.reshape(1, 500), (BLOC, 1)).astype(np.float32)
    consts["uw_exp"] = np.tile(user_w.reshape(1, 50), (BLOC, 1)).astype(np.float32)
    consts["iw_exp"] = np.tile(item_w.reshape(1, 50), (BLOC, 1)).astype(np.float32)

    in_maps = []
    for c in range(NCORE):
        uids = U_ids[c * BLOC:(c + 1) * BLOC]
        iids = I_ids[c * BLOC:(c + 1) * BLOC]
        m = dict(consts)
        m["u_ids"] = uids.astype(np.int32)[:, None].copy()
        m["i_ids"] = iids.astype(np.int32)[:, None].copy()
        m["u_idx16"], m["u_par"] = _idx_layout(uids, U_docs)
        m["i_idx16"], m["i_par"] = _idx_layout(iids, I_docs)
        m["my_shard"] = words_pad[c * SHARD:(c + 1) * SHARD]
        in_maps.append(m)

    if "nc" not in _NC_CACHE:
        _NC_CACHE["nc"] = _build_nc()
    nc = _NC_CACHE["nc"]
    global _LAST_IN_MAPS
    _LAST_IN_MAPS = in_maps

    res = run_bass_kernel_spmd(nc, in_maps, core_ids=list(range(NCORE)))
    out = np.concatenate([np.asarray(res.results[c]["out"]).reshape(BLOC)
                          for c in range(NCORE)])
    return out.astype(np.float32)



# revision 4
# speedup vs baseline: 1.0322x; 1.0322x over previous
"""ANR sparse-attention recommender on 8 Trainium2 NeuronCores.

Strategy (data-parallel on batch, vocab-sharded pre-projection):
  P1: each core projects its 1/8 vocab shard through PROJ_EXT [300,64]
      (cols 0..49 = per-aspect projection (a,h)-major; cols 50..59 = the
      w=0 and w=2 window weights folded in; center w=1 computed on device)
  P2: AllGather the projected table gtab [50176, 64] f32 (12.8MB)
  P3: per-core dma_gather of PAIR rows (gtab viewed [25088, 128], 512B elem,
      idx16 = token_id//2 host-laid-out in the Q7 16-partition wrap) +
      parity select on DVE.  Token slot j -> (partition j%128, col j//128);
      partition p = 4*item + quarter, col t -> l = 125*quarter + t.
  P4: center logit via DVE mult+reduce; window shifts along the free dim
      (+ PE shift-matrix edge fixups); softmax over l via free-reduce +
      selector-matmul cross-quarter sum; rep = attn-weighted reduce +
      selector-matmul; tiny co-attention on DVE with host-expanded weights.
"""
import numpy as np

import concourse.bass as bass
import concourse.bacc as bacc
import concourse.mybir as mybir
import concourse.tile as tile
from concourse.bass_utils import run_bass_kernel_spmd

A, L, D, H1, H2, CWS = 5, 500, 300, 10, 50, 3
V, NU, NI, B = 50000, 20000, 20000, 256
NCORE, BLOC = 8, 32
SHARD = 6272                 # per-core vocab rows (padded); 8*6272 = 50176
VPAD = SHARD * NCORE
GCOL = 64                    # gtab row: 50 adoc + 5 g0 + 5 g2 + 4 pad
NT = SHARD // 128            # 49 tiles per shard
NTOK = 16000                 # tokens per side per core (32 items x 500)
F32 = mybir.dt.float32
I32 = mybir.dt.int32
I16 = mybir.dt.int16
I8 = mybir.dt.uint8
DCH = [(0, 128), (128, 128), (256, 44)]   # D=300 chunks
MUL = mybir.AluOpType.mult
ADD = mybir.AluOpType.add


def _build_nc():
    nc = bacc.Bacc(num_swdge_queues=4)
    P = nc.declare_dram_parameter

    u_ids = P("u_ids", [BLOC, 1], I32, isOutput=False)
    i_ids = P("i_ids", [BLOC, 1], I32, isOutput=False)
    u_idx16 = P("u_idx16", [128, 1000], I16, isOutput=False)
    i_idx16 = P("i_idx16", [128, 1000], I16, isOutput=False)
    u_par = P("u_par", [128, 125], I8, isOutput=False)
    i_par = P("i_par", [128, 125], I8, isOutput=False)
    my_shard = P("my_shard", [SHARD, D], F32, isOutput=False)
    pext = P("pext", [D, GCOL], F32, isOutput=False)
    ident = P("ident", [128, 128], F32, isOutput=False)
    p4sel = P("p4sel", [128, BLOC], F32, isOutput=False)
    p4selT = P("p4selT", [BLOC, 128], F32, isOutput=False)
    shdn = P("shdn", [128, 128], F32, isOutput=False)   # out[m]=in[m-1] if m%4!=0
    shup = P("shup", [128, 128], F32, isOutput=False)   # out[m]=in[m+1] if m%4!=3
    e1c = P("e1c", [128, 50], F32, isOutput=False)      # E[a, 10+h] all partitions
    m_exp = P("m_exp", [BLOC, 100], F32, isOutput=False)
    upT_exp = P("upT_exp", [BLOC, 500], F32, isOutput=False)
    ipT_exp = P("ipT_exp", [BLOC, 500], F32, isOutput=False)
    uw_exp = P("uw_exp", [BLOC, 50], F32, isOutput=False)
    iw_exp = P("iw_exp", [BLOC, 50], F32, isOutput=False)
    bu = P("bu", [NU, 1], F32, isOutput=False)
    bi = P("bi", [NI, 1], F32, isOutput=False)
    bg = P("bg", [BLOC, 1], F32, isOutput=False)
    out_ext = P("out", [BLOC, 1], F32, isOutput=True)

    with tile.TileContext(nc) as tc:
        with (
            tc.tile_pool(name="dram", bufs=1, space="DRAM") as DP,
            tc.tile_pool(name="consts", bufs=1) as CP,
            tc.tile_pool(name="p1", bufs=4) as P1,
            tc.tile_pool(name="ps", bufs=1, space="PSUM") as PS,
            tc.tile_pool(name="big", bufs=1) as BG,
            tc.tile_pool(name="work", bufs=2) as WK,
            tc.tile_pool(name="scr", bufs=4) as SC,
        ):
            pv_shard = DP.tile([SHARD, GCOL], F32)
            gtab = DP.tile([VPAD, GCOL], F32, addr_space="Shared")

            # ---- load constants ----
            ident_sb = CP.tile([128, 128], F32)
            nc.sync.dma_start(out=ident_sb[:], in_=ident[:])
            p4sel_sb = CP.tile([128, BLOC], F32)
            nc.sync.dma_start(out=p4sel_sb[:], in_=p4sel[:])
            p4selT_sb = CP.tile([BLOC, 128], F32)
            nc.sync.dma_start(out=p4selT_sb[:], in_=p4selT[:])
            shdn_sb = CP.tile([128, 128], F32)
            nc.sync.dma_start(out=shdn_sb[:], in_=shdn[:])
            shup_sb = CP.tile([128, 128], F32)
            nc.sync.dma_start(out=shup_sb[:], in_=shup[:])
            e1c_sb = CP.tile([128, 50], F32)
            nc.sync.dma_start(out=e1c_sb[:], in_=e1c[:])
            pext_sb = []
            for c, (d0, dn) in enumerate(DCH):
                t = CP.tile([128, GCOL], F32, name=f"pext{c}")
                nc.sync.dma_start(out=t[:dn, :], in_=pext[d0:d0 + dn, :])
                pext_sb.append(t)

            # ---- P1: project vocab shard (4 tiles per load) ----
            for g in range((NT + 3) // 4):
                tlo = g * 4
                thi = min(tlo + 4, NT)
                ng = thi - tlo
                emb = P1.tile([128, 4 * D], F32, tag="emb", bufs=2)
                nc.sync.dma_start(
                    out=emb[:, 0:ng * D],
                    in_=my_shard[tlo * 128:thi * 128, :]
                        .rearrange("(c p) d -> p c d", p=128))
                for t in range(tlo, thi):
                    cc = (t - tlo) * D
                    embT = []
                    for c, (d0, dn) in enumerate(DCH):
                        tp = PS.tile([128, 128], F32, tag="tp", bufs=3)
                        nc.tensor.transpose(out=tp[:dn, :],
                                            in_=emb[:, cc + d0:cc + d0 + dn],
                                            identity=ident_sb[:])
                        eT = P1.tile([128, 128], F32, tag=f"embT{c}")
                        nc.vector.tensor_copy(out=eT[:dn, :], in_=tp[:dn, :])
                        embT.append(eT)
                    pvo = PS.tile([128, GCOL], F32, tag="pvo", bufs=3)
                    for c, (d0, dn) in enumerate(DCH):
                        nc.tensor.matmul(out=pvo[:], lhsT=embT[c][:dn, :],
                                         rhs=pext_sb[c][:dn, :],
                                         start=(c == 0), stop=(c == 2))
                    pvs = P1.tile([128, GCOL], F32, tag="pvs")
                    nc.scalar.copy(out=pvs[:], in_=pvo[:])
                    nc.sync.dma_start(out=pv_shard[t * 128:(t + 1) * 128, :],
                                      in_=pvs[:])

            # ---- P2: AllGather ----
            nc.gpsimd.collective_compute(
                "AllGather", mybir.AluOpType.bypass,
                replica_groups=[list(range(NCORE))],
                ins=[pv_shard[:].opt()], outs=[gtab[:].opt()],
            )
            gtab_pairs_u16 = gtab[:].bitcast(mybir.dt.uint16) \
                                    .rearrange("(v two) e -> v (two e)", two=2)

            # ---- P3+P4 per side ----
            reps = {}
            for side, (idx_p, par_p) in {"u": (u_idx16, u_par),
                                         "i": (i_idx16, i_par)}.items():
                idx_sb = WK.tile([128, 1000], I16, tag="idx")
                nc.sync.dma_start(out=idx_sb[:], in_=idx_p[:])
                par_sb = WK.tile([128, 125], I8, tag="par")
                nc.sync.dma_start(out=par_sb[:], in_=par_p[:])

                # gather in 16-t-block chunks (2048 idxs, single_packet)
                sel = BG.tile([128, 125 * GCOL], F32, tag="sel", bufs=2)
                sel3 = sel[:].rearrange("p (t e) -> p t e", e=GCOL)
                bounds = [(0, 32), (32, 63), (63, 95), (95, 125)]
                for qi, (t0, t1) in enumerate(bounds):
                    nt = (t1 - t0) * 128
                    gr = BG.tile([128, 32 * 128], F32, tag="gr", bufs=2)
                    gr3 = gr[:].rearrange("p (t e) -> p t e", e=128)
                    gr3u = gr[:].bitcast(mybir.dt.uint16) \
                                .rearrange("p (t e) -> p t e", e=256)
                    nc.gpsimd.dma_gather(
                        out_ap=gr3u[:, 0:t1 - t0, :], in_ap=gtab_pairs_u16,
                        idxs_ap=idx_sb[:, t0 * 8:t1 * 8],
                        num_idxs=nt, num_idxs_reg=nt, elem_size=256,
                        single_packet=False, queue_num=qi)
                    # parity select: sel[p,t,:] = gr[p,t, parity*64 : +64]
                    nc.scalar.copy(out=sel3[:, t0:t1, :],
                                   in_=gr3[:, 0:t1 - t0, 0:GCOL])
                    mask3 = par_sb[:, t0:t1].unsqueeze(2) \
                                            .to_broadcast([128, t1 - t0, GCOL])
                    nc.vector.copy_predicated(out=sel3[:, t0:t1, :], mask=mask3,
                                              data=gr3[:, 0:t1 - t0, GCOL:2 * GCOL])
                adoc = sel3[:, :, 0:50].rearrange("p t (a h) -> p t a h", a=A)
                g0s = sel3[:, :, 50:55]     # [p, t, a]
                g2s = sel3[:, :, 55:60]

                # center logit lgc[p,t,a] = sum_h adoc * E1
                wct = BG.tile([128, 6250], F32, tag="wad", bufs=1)
                wct4 = wct[:].rearrange("p (t a h) -> p t a h", a=A, h=H1)
                e1b = e1c_sb[:].rearrange("p (a h) -> p a h", a=A) \
                               .unsqueeze(1).to_broadcast([128, 125, A, H1])
                nc.vector.tensor_tensor(out=wct4, in0=adoc, in1=e1b, op=MUL)
                lg = WK.tile([128, 625], F32, tag="lg")     # [p, t, a]
                lg3 = lg[:].rearrange("p (t a) -> p t a", a=A)
                nc.vector.tensor_reduce(out=lg3, in_=wct4,
                                        axis=mybir.AxisListType.X,
                                        op=mybir.AluOpType.add)
                # window shifts along t
                nc.vector.tensor_tensor(out=lg3[:, 1:125, :], in0=lg3[:, 1:125, :],
                                        in1=g0s[:, 0:124, :], op=ADD)
                nc.vector.tensor_tensor(out=lg3[:, 0:124, :], in0=lg3[:, 0:124, :],
                                        in1=g2s[:, 1:125, :], op=ADD)
                # cross-quarter edges via PE shift matrices
                e0 = PS.tile([128, A], F32, tag="sps", bufs=2)
                nc.tensor.matmul(out=e0[:], lhsT=shdn_sb[:], rhs=g0s[:, 124, :],
                                 start=True, stop=True)
                nc.vector.tensor_tensor(out=lg3[:, 0, :], in0=lg3[:, 0, :],
                                        in1=e0[:], op=ADD)
                e1m = PS.tile([128, A], F32, tag="sps", bufs=2)
                nc.tensor.matmul(out=e1m[:], lhsT=shup_sb[:], rhs=g2s[:, 0, :],
                                 start=True, stop=True)
                nc.vector.tensor_tensor(out=lg3[:, 124, :], in0=lg3[:, 124, :],
                                        in1=e1m[:], op=ADD)

                # softmax over l (no max shift; logits are tiny)
                E = WK.tile([128, 625], F32, tag="E")
                nc.scalar.activation(out=E[:], in_=lg[:],
                                     func=mybir.ActivationFunctionType.Exp)
                E3 = E[:].rearrange("p (t a) -> p t a", a=A)
                Eat = E[:].rearrange("p (t a) -> p a t", a=A)
                S = SC.tile([128, A], F32, tag="S")
                nc.vector.tensor_reduce(out=S[:], in_=Eat,
                                        axis=mybir.AxisListType.X,
                                        op=mybir.AluOpType.add)
                sit = PS.tile([BLOC, A], F32, tag="sps", bufs=2)
                nc.tensor.matmul(out=sit[:], lhsT=p4sel_sb[:], rhs=S[:],
                                 start=True, stop=True)
                srec = SC.tile([BLOC, A], F32, tag="srec")
                nc.vector.reciprocal(out=srec[:], in_=sit[:])
                sbc = PS.tile([128, A], F32, tag="sps", bufs=2)
                nc.tensor.matmul(out=sbc[:], lhsT=p4selT_sb[:], rhs=srec[:],
                                 start=True, stop=True)
                attn = WK.tile([128, 625], F32, tag="attn")
                attn3 = attn[:].rearrange("p (t a) -> p t a", a=A)
                sbc3 = sbc[:].unsqueeze(1).to_broadcast([128, 125, A])
                nc.vector.tensor_tensor(out=attn3, in0=E3, in1=sbc3, op=MUL)

                # rep: weighted sum of adoc over l, then cross-quarter sum
                wad = BG.tile([128, 6250], F32, tag="wad", bufs=1)
                wad4 = wad[:].rearrange("p (t a h) -> p t a h", a=A, h=H1)
                attnb = attn3.unsqueeze(3).to_broadcast([128, 125, A, H1])
                nc.vector.tensor_tensor(out=wad4, in0=adoc, in1=attnb, op=MUL)
                wsum = WK.tile([128, 50], F32, tag="wsum")
                wad_aht = wad[:].rearrange("p (t ah) -> p ah t", ah=50)
                nc.vector.tensor_reduce(out=wsum[:], in_=wad_aht,
                                        axis=mybir.AxisListType.X,
                                        op=mybir.AluOpType.add)
                repp = PS.tile([BLOC, 50], F32, tag="sps", bufs=2)
                nc.tensor.matmul(out=repp[:], lhsT=p4sel_sb[:], rhs=wsum[:],
                                 start=True, stop=True)
                rep = WK.tile([BLOC, 50], F32, tag=f"rep_{side}")
                nc.vector.tensor_copy(out=rep[:], in_=repp[:])
                reps[side] = rep

            # ---- co-attention (all [32, *] DVE ops) ----
            mexp_sb = CP.tile([BLOC, 100], F32)
            nc.sync.dma_start(out=mexp_sb[:], in_=m_exp[:])
            up_sb = CP.tile([BLOC, 500], F32)
            nc.sync.dma_start(out=up_sb[:], in_=upT_exp[:])
            ip_sb = CP.tile([BLOC, 500], F32)
            nc.sync.dma_start(out=ip_sb[:], in_=ipT_exp[:])
            uw_sb = CP.tile([BLOC, 50], F32)
            nc.sync.dma_start(out=uw_sb[:], in_=uw_exp[:])
            iw_sb = CP.tile([BLOC, 50], F32)
            nc.sync.dma_start(out=iw_sb[:], in_=iw_exp[:])

            ru, ri = reps["u"][:], reps["i"][:]
            ru3 = ru.rearrange("p (a h) -> p a h", a=A)     # [32, 5, 10]
            ri3 = ri.rearrange("p (c k) -> p c k", c=A)
            mexp3 = mexp_sb[:].rearrange("p (h k) -> p h k", h=H1)

            # UdM[b,(a,k)] = sum_h Ud[b,(a,h)] * M[h,k]
            UdM = WK.tile([BLOC, 50], F32, tag="UdM")
            UdM3 = UdM[:].rearrange("p (a k) -> p a k", a=A)
            s50 = SC.tile([BLOC, 50], F32, tag="s50")
            s50_3 = s50[:].rearrange("p (a k) -> p a k", a=A)
            for h in range(H1):
                in0 = ru3[:, :, h].unsqueeze(2).to_broadcast([BLOC, A, H1])
                in1 = mexp3[:, h, :].unsqueeze(1).to_broadcast([BLOC, A, H1])
                nc.vector.tensor_tensor(out=(UdM3 if h == 0 else s50_3),
                                        in0=in0, in1=in1, op=MUL)
                if h > 0:
                    nc.vector.tensor_tensor(out=UdM[:], in0=UdM[:], in1=s50[:], op=ADD)
            # aff[b,(a,c)] = relu(sum_k UdM[b,(a,k)] * Id[b,(c,k)])
            aff0 = WK.tile([BLOC, 25], F32, tag="aff0")
            aff0_3 = aff0[:].rearrange("p (a c) -> p a c", a=A)
            s25 = SC.tile([BLOC, 25], F32, tag="s25")
            s25_3 = s25[:].rearrange("p (a c) -> p a c", a=A)
            for k in range(H1):
                in0 = UdM3[:, :, k].unsqueeze(2).to_broadcast([BLOC, A, A])
                in1 = ri3[:, :, k].unsqueeze(1).to_broadcast([BLOC, A, A])
                nc.vector.tensor_tensor(out=(aff0_3 if k == 0 else s25_3),
                                        in0=in0, in1=in1, op=MUL)
                if k > 0:
                    nc.vector.tensor_tensor(out=aff0[:], in0=aff0[:], in1=s25[:], op=ADD)
            aff = WK.tile([BLOC, 25], F32, tag="aff")
            nc.vector.tensor_scalar_max(out=aff[:], in0=aff0[:], scalar1=0.0)
            aff3 = aff[:].rearrange("p (a c) -> p a c", a=A)

            # Hu1[b,(e,a)] = sum_h up[e,h] Ud[b,(a,h)];  Hi1 likewise
            def proj_h(dst, w_sb, r3):
                dst3 = dst[:].rearrange("p (e a) -> p e a", e=H2)
                s250 = SC.tile([BLOC, 250], F32, tag="s250")
                s250_3 = s250[:].rearrange("p (e a) -> p e a", e=H2)
                w3 = w_sb[:].rearrange("p (h e) -> p h e", h=H1)
                for h in range(H1):
                    in0 = r3[:, :, h].unsqueeze(1).to_broadcast([BLOC, H2, A])
                    in1 = w3[:, h, :].unsqueeze(2).to_broadcast([BLOC, H2, A])
                    nc.vector.tensor_tensor(out=(dst3 if h == 0 else s250_3),
                                            in0=in0, in1=in1, op=MUL)
                    if h > 0:
                        nc.vector.tensor_tensor(out=dst[:], in0=dst[:],
                                                in1=s250[:], op=ADD)

            Hu1 = WK.tile([BLOC, 250], F32, tag="Hu1")
            proj_h(Hu1, up_sb, ru3)
            Hi1 = WK.tile([BLOC, 250], F32, tag="Hi1")
            proj_h(Hi1, ip_sb, ri3)

            # Hu = relu(Hu1 + sum_c Hi1[b,(e,c)] aff[b,(a,c)])
            # Hi = relu(Hi1 + sum_a Hu1[b,(e,a)] aff[b,(a,c)])
            def coatt(dst, h1_self, h1_other, sum_over_c):
                acc = WK.tile([BLOC, 250], F32, tag=f"acc_{sum_over_c}")
                h1o3 = h1_other[:].rearrange("p (e x) -> p e x", e=H2)
                s250b = SC.tile([BLOC, 250], F32, tag="s250b")
                for c in range(A):
                    in0 = h1o3[:, :, c].unsqueeze(2).to_broadcast([BLOC, H2, A])
                    if sum_over_c:   # out index a; aff[:, a, c]
                        in1 = aff3[:, :, c].unsqueeze(1).to_broadcast([BLOC, H2, A])
                    else:            # out index c'; aff[:, c(=a), c']
                        in1 = aff3[:, c, :].unsqueeze(1).to_broadcast([BLOC, H2, A])
                    nc.vector.tensor_tensor(
                        out=s250b[:].rearrange("p (e a) -> p e a", e=H2),
                        in0=in0, in1=in1, op=MUL)
                    src = h1_self[:] if c == 0 else acc[:]
                    nc.vector.tensor_tensor(out=acc[:], in0=src, in1=s250b[:], op=ADD)
                nc.vector.tensor_scalar_max(out=dst[:], in0=acc[:], scalar1=0.0)

            Hu = WK.tile([BLOC, 250], F32, tag="Hu")
            coatt(Hu, Hu1, Hi1, sum_over_c=True)
            Hi = WK.tile([BLOC, 250], F32, tag="Hi")
            coatt(Hi, Hi1, Hu1, sum_over_c=False)

            # imp logits lu[b,a] = sum_e uw[e] Hu[b,(e,a)]
            def imp(dst5, Hx, wx_sb):
                s250c = SC.tile([BLOC, 250], F32, tag="s250c")
                nc.vector.tensor_tensor(
                    out=s250c[:].rearrange("p (e a) -> p e a", e=H2),
                    in0=Hx[:].rearrange("p (e a) -> p e a", e=H2),
                    in1=wx_sb[:].unsqueeze(2).to_broadcast([BLOC, H2, A]), op=MUL)
                v = s250c[:].rearrange("p (e a) -> p a e", e=H2)
                nc.vector.tensor_reduce(out=dst5, in_=v, axis=mybir.AxisListType.X,
                                        op=mybir.AluOpType.add)

            lu = SC.tile([BLOC, A], F32, tag="lu")
            imp(lu[:], Hu, uw_sb)
            li = SC.tile([BLOC, A], F32, tag="li")
            imp(li[:], Hi, iw_sb)
            eu = SC.tile([BLOC, A], F32, tag="eu")
            nc.scalar.activation(out=eu[:], in_=lu[:],
                                 func=mybir.ActivationFunctionType.Exp)
            ei = SC.tile([BLOC, A], F32, tag="ei")
            nc.scalar.activation(out=ei[:], in_=li[:],
                                 func=mybir.ActivationFunctionType.Exp)
            su = SC.tile([BLOC, 1], F32, tag="su")
            nc.vector.tensor_reduce(out=su[:], in_=eu[:], axis=mybir.AxisListType.X,
                                    op=mybir.AluOpType.add)
            si = SC.tile([BLOC, 1], F32, tag="si")
            nc.vector.tensor_reduce(out=si[:], in_=ei[:], axis=mybir.AxisListType.X,
                                    op=mybir.AluOpType.add)
            sur = SC.tile([BLOC, 1], F32, tag="sur")
            nc.vector.reciprocal(out=sur[:], in_=su[:])
            sir = SC.tile([BLOC, 1], F32, tag="sir")
            nc.vector.reciprocal(out=sir[:], in_=si[:])

            # ar[b,a] = sum_h Ud*Id
            arm = SC.tile([BLOC, 50], F32, tag="arm")
            nc.vector.tensor_tensor(out=arm[:], in0=ru, in1=ri, op=MUL)
            ar5 = SC.tile([BLOC, A], F32, tag="ar5")
            nc.vector.tensor_reduce(out=ar5[:],
                                    in_=arm[:].rearrange("p (a h) -> p a h", a=A),
                                    axis=mybir.AxisListType.X, op=mybir.AluOpType.add)
            # R = sum_a eu*ei*ar / (su*si) + biases
            pr = SC.tile([BLOC, A], F32, tag="pr")
            nc.vector.tensor_tensor(out=pr[:], in0=eu[:], in1=ei[:], op=MUL)
            nc.vector.tensor_tensor(out=pr[:], in0=pr[:], in1=ar5[:], op=MUL)
            r0 = SC.tile([BLOC, 1], F32, tag="r0")
            nc.vector.tensor_reduce(out=r0[:], in_=pr[:], axis=mybir.AxisListType.X,
                                    op=mybir.AluOpType.add)
            nc.vector.tensor_tensor(out=r0[:], in0=r0[:], in1=sur[:], op=MUL)
            nc.vector.tensor_tensor(out=r0[:], in0=r0[:], in1=sir[:], op=MUL)

            uid_sb = SC.tile([BLOC, 1], I32, tag="uid")
            nc.sync.dma_start(out=uid_sb[:], in_=u_ids[:])
            iid_sb = SC.tile([BLOC, 1], I32, tag="iid")
            nc.sync.dma_start(out=iid_sb[:], in_=i_ids[:])
            bu_g = SC.tile([BLOC, 1], F32, tag="bu_g")
            nc.gpsimd.indirect_dma_start(
                out=bu_g[:], out_offset=None, in_=bu[:],
                in_offset=bass.IndirectOffsetOnAxis(ap=uid_sb[:, :1], axis=0))
            bi_g = SC.tile([BLOC, 1], F32, tag="bi_g")
            nc.gpsimd.indirect_dma_start(
                out=bi_g[:], out_offset=None, in_=bi[:],
                in_offset=bass.IndirectOffsetOnAxis(ap=iid_sb[:, :1], axis=0))
            bg_sb = SC.tile([BLOC, 1], F32, tag="bg_sb")
            nc.sync.dma_start(out=bg_sb[:], in_=bg[:])
            nc.vector.tensor_tensor(out=r0[:], in0=r0[:], in1=bu_g[:], op=ADD)
            nc.vector.tensor_tensor(out=r0[:], in0=r0[:], in1=bi_g[:], op=ADD)
            nc.vector.tensor_tensor(out=r0[:], in0=r0[:], in1=bg_sb[:], op=ADD)
            nc.sync.dma_start(out=out_ext[:], in_=r0[:])

    nc.finalize()
    return nc


_NC_CACHE = {}
_LAST_IN_MAPS = None


def _idx_layout(ids, docs):
    """idx16 [128,1000] int16 (pair idx) + parity [128,125] f32 for one side."""
    j = np.arange(NTOK)
    p = j % 128
    t = j // 128
    item = p // 4
    l = 125 * (p % 4) + t
    tok = docs[ids[item], l].astype(np.int64)          # [NTOK]
    blk = np.zeros((16, 1000), np.int16)
    blk[j % 16, j // 16] = (tok // 2).astype(np.int16)
    idx16 = np.tile(blk, (8, 1))       # replicated across the 8 Q7 cores
    par = np.zeros((128, 125), np.uint8)
    par[p, t] = (tok % 2).astype(np.uint8)
    return idx16, par


def kernel(U_ids, I_ids, U_docs, I_docs, words_emb, aspect_emb, aspect_proj,
           M, user_proj, user_w, item_proj, item_w, Bu, Bi, Bg):
    U_ids = np.asarray(U_ids).astype(np.int64).reshape(B)
    I_ids = np.asarray(I_ids).astype(np.int64).reshape(B)
    U_docs = np.asarray(U_docs).astype(np.int64)
    I_docs = np.asarray(I_docs).astype(np.int64)
    words_emb = np.asarray(words_emb, np.float32)
    aspect_emb = np.asarray(aspect_emb, np.float32)
    aspect_proj = np.asarray(aspect_proj, np.float32)
    M = np.asarray(M, np.float32)
    user_proj = np.asarray(user_proj, np.float32)
    user_w = np.asarray(user_w, np.float32)
    item_proj = np.asarray(item_proj, np.float32)
    item_w = np.asarray(item_w, np.float32)
    Bu = np.asarray(Bu, np.float32); Bi = np.asarray(Bi, np.float32)
    Bg = np.float32(np.asarray(Bg))

    # ---- host-side parameter prep ----
    pext = np.zeros((D, GCOL), np.float32)
    for a in range(A):
        pext[:, a * 10:(a + 1) * 10] = aspect_proj[a]
    for a in range(A):
        pext[:, 50 + a] = aspect_proj[a] @ aspect_emb[a, 0:10]        # g0 (w=0)
        pext[:, 55 + a] = aspect_proj[a] @ aspect_emb[a, 20:30]       # g2 (w=2)

    words_pad = np.zeros((VPAD, D), np.float32)
    words_pad[:V] = words_emb

    pr = np.arange(128)
    e1 = np.empty((128, 50), np.float32)
    for a in range(A):
        e1[:, a * 10:(a + 1) * 10] = aspect_emb[a, 10:20][None, :]
    consts = {
        "ident": np.eye(128, dtype=np.float32),
        "p4sel": (pr[:, None] // 4 == np.arange(BLOC)[None, :]).astype(np.float32),
        "p4selT": (pr[None, :] // 4 == np.arange(BLOC)[:, None]).astype(np.float32),
        "shdn": ((pr[None, :] == pr[:, None] + 1) &
                 (pr[None, :] % 4 != 0)).astype(np.float32),
        "shup": ((pr[None, :] == pr[:, None] - 1) &
                 (pr[None, :] % 4 != 3)).astype(np.float32),
        "e1c": e1,
        "pext": pext,
        "bu": Bu[:, None].copy(), "bi": Bi[:, None].copy(),
        "bg": np.full((BLOC, 1), Bg, np.float32),
    }
    consts["m_exp"] = np.tile(M.reshape(1, 100), (BLOC, 1)).astype(np.float32)
    consts["upT_exp"] = np.tile(user_proj.T.reshape(1, 500), (BLOC, 1)).astype(np.float32)
    consts["ipT_exp"] = np.tile(item_proj.T.reshape(1, 500), (BLOC, 1)).astype(np.float32)
    consts["uw_exp"] = np.tile(user_w.reshape(1, 50), (BLOC, 1)).astype(np.float32)
    consts["iw_exp"] = np.tile(item_w.reshape(1, 50), (BLOC, 1)).astype(np.float32)

    in_maps = []
    for c in range(NCORE):
        uids = U_ids[c * BLOC:(c + 1) * BLOC]
        iids = I_ids[c * BLOC:(c + 1) * BLOC]
        m = dict(consts)
        m["u_ids"] = uids.astype(np.int32)[:, None].copy()
        m["i_ids"] = iids.astype(np.int32)[:, None].copy()
        m["u_idx16"], m["u_par"] = _idx_layout(uids, U_docs)
        m["i_idx16"], m["i_par"] = _idx_layout(iids, I_docs)
        m["my_shard"] = words_pad[c * SHARD:(c + 1) * SHARD]
        in_maps.append(m)

    if "nc" not in _NC_CACHE:
        _NC_CACHE["nc"] = _build_nc()
    nc = _NC_CACHE["nc"]
    global _LAST_IN_MAPS
    _LAST_IN_MAPS = in_maps

    res = run_bass_kernel_spmd(nc, in_maps, core_ids=list(range(NCORE)))
    out = np.concatenate([np.asarray(res.results[c]["out"]).reshape(BLOC)
                          for c in range(NCORE)])
    return out.astype(np.float32)



# revision 8
# speedup vs baseline: 1.7056x; 1.6524x over previous
"""ANR sparse-attention recommender on 8 Trainium2 NeuronCores.

Strategy (data-parallel on batch, vocab-sharded pre-projection, bf16):
  P1: each core projects its 1/8 vocab shard (host-transposed bf16
      [300, 6272]) through PEXT [300,64] bf16 via straight PE matmuls
      (no on-device transposes) -> pv_shard [6272, 64] bf16.
  P2: AllGather the projected table gtab [50176, 64] bf16 (6.4MB).
  P3: 8 dma_gather PREPS (4 SWDGE queues x 2 sides) generate descriptors
      on the Q7 pairs CONCURRENTLY while P1+AllGather run; trigger_dma
      fires the transfers as soon as gtab lands.  Pair rows (gtab viewed
      [25088, 128] bf16, 256B elem, idx16 = token_id//2) + parity select.
      Token slot j -> (partition j%128, col j//128); partition
      p = 4*item + quarter, col t -> l = 125*quarter + t.
  P4: center logit via DVE bf16 mult + f32 reduce; window shifts along
      the free dim (+ PE shift-matrix edge fixups); softmax over l via
      free-reduce + selector-matmul cross-quarter sum; rep = attn-
      weighted bf16 reduce + selector-matmul; co-attention on DVE f32.
  Bias Bu[uid]+Bi[iid]+Bg folded on host (parameter prep).
"""
import numpy as np
import ml_dtypes

import concourse.bass as bass
import concourse.bacc as bacc
import concourse.mybir as mybir
import concourse.tile as tile
from concourse.bass_utils import run_bass_kernel_spmd

A, L, D, H1, H2, CWS = 5, 500, 300, 10, 50, 3
V, NU, NI, B = 50000, 20000, 20000, 256
NCORE, BLOC = 8, 32
SHARD = 6272                 # per-core vocab rows (padded); 8*6272 = 50176
VPAD = SHARD * NCORE
GCOL = 64                    # gtab row: 50 adoc + 5 g0 + 5 g2 + 4 pad
NT = SHARD // 128            # 49 tiles per shard
NTOK = 16000                 # tokens per side per core (32 items x 500)
F32 = mybir.dt.float32
BF16 = mybir.dt.bfloat16
I32 = mybir.dt.int32
I16 = mybir.dt.int16
I8 = mybir.dt.uint8
U16 = mybir.dt.uint16
DCH = [(0, 128), (128, 128), (256, 44)]   # D=300 chunks
MUL = mybir.AluOpType.mult
ADD = mybir.AluOpType.add
BOUNDS = [(0, 32), (32, 63), (63, 95), (95, 125)]


def _build_nc():
    nc = bacc.Bacc(num_swdge_queues=4)
    P = nc.declare_dram_parameter

    u_idx16 = P("u_idx16", [128, 1000], I16, isOutput=False)
    i_idx16 = P("i_idx16", [128, 1000], I16, isOutput=False)
    u_par = P("u_par", [128, 125], I8, isOutput=False)
    i_par = P("i_par", [128, 125], I8, isOutput=False)
    my_shardT = P("my_shardT", [D, SHARD], BF16, isOutput=False)
    pext = P("pext", [D, GCOL], BF16, isOutput=False)
    p4sel = P("p4sel", [128, BLOC], F32, isOutput=False)
    p4selT = P("p4selT", [BLOC, 128], F32, isOutput=False)
    shdn = P("shdn", [128, 128], F32, isOutput=False)   # out[m]=in[m-1] if m%4!=0
    shup = P("shup", [128, 128], F32, isOutput=False)   # out[m]=in[m+1] if m%4!=3
    e1c = P("e1c", [128, 50], BF16, isOutput=False)     # E[a, 10+h] all partitions
    m_exp = P("m_exp", [BLOC, 100], F32, isOutput=False)
    upT_exp = P("upT_exp", [BLOC, 500], F32, isOutput=False)
    ipT_exp = P("ipT_exp", [BLOC, 500], F32, isOutput=False)
    uw_exp = P("uw_exp", [BLOC, 50], F32, isOutput=False)
    iw_exp = P("iw_exp", [BLOC, 50], F32, isOutput=False)
    bias = P("bias", [BLOC, 1], F32, isOutput=False)    # Bu[uid]+Bi[iid]+Bg
    out_ext = P("out", [BLOC, 1], F32, isOutput=True)

    with tile.TileContext(nc) as tc:
        with (
            tc.tile_pool(name="dram", bufs=1, space="DRAM") as DP,
            tc.tile_pool(name="consts", bufs=1) as CP,
            tc.tile_pool(name="shard", bufs=1) as ST,
            tc.tile_pool(name="gr", bufs=1) as GR,
            tc.tile_pool(name="ps", bufs=1, space="PSUM") as PS,
            tc.tile_pool(name="big", bufs=1) as BG,
            tc.tile_pool(name="work", bufs=2) as WK,
            tc.tile_pool(name="scr", bufs=2) as SC,
        ):
            pv_shard = DP.tile([SHARD, GCOL], BF16)
            gtab = DP.tile([VPAD, GCOL], BF16, addr_space="Shared")
            gtab_pairs = gtab[:].bitcast(U16) \
                                .rearrange("(v two) e -> v (two e)", two=2)

            # ---- idx/par loads first (gather preps depend on them) ----
            idx_sb, par_sb = {}, {}
            for side, (idx_p, par_p) in (("u", (u_idx16, u_par)),
                                         ("i", (i_idx16, i_par))):
                t = WK.tile([128, 1000], I16, tag=f"idx_{side}", bufs=1)
                nc.sync.dma_start(out=t[:], in_=idx_p[:])
                idx_sb[side] = t
                t = WK.tile([128, 125], I8, tag=f"par_{side}", bufs=1)
                nc.sync.dma_start(out=t[:], in_=par_p[:])
                par_sb[side] = t

            # ---- constants ----
            p4sel_sb = CP.tile([128, BLOC], F32)
            nc.sync.dma_start(out=p4sel_sb[:], in_=p4sel[:])
            p4selT_sb = CP.tile([BLOC, 128], F32)
            nc.sync.dma_start(out=p4selT_sb[:], in_=p4selT[:])
            shdn_sb = CP.tile([128, 128], F32)
            nc.sync.dma_start(out=shdn_sb[:], in_=shdn[:])
            shup_sb = CP.tile([128, 128], F32)
            nc.sync.dma_start(out=shup_sb[:], in_=shup[:])
            e1c_sb = CP.tile([128, 50], BF16)
            nc.sync.dma_start(out=e1c_sb[:], in_=e1c[:])
            pext_sb = []
            for c, (d0, dn) in enumerate(DCH):
                t = CP.tile([128, GCOL], BF16, name=f"pext{c}")
                nc.sync.dma_start(out=t[:dn, :], in_=pext[d0:d0 + dn, :])
                pext_sb.append(t)
            st_sb = []
            for c, (d0, dn) in enumerate(DCH):
                t = ST.tile([128, SHARD], BF16, tag=f"st{c}")
                nc.sync.dma_start(out=t[:dn, :], in_=my_shardT[d0:d0 + dn, :])
                st_sb.append(t)

            # ---- P1: project vocab shard (PE only, no transposes) ----
            for t in range(NT):
                pvo = PS.tile([128, GCOL], F32, tag="pvo", bufs=4)
                for c, (d0, dn) in enumerate(DCH):
                    nc.tensor.matmul(out=pvo[:],
                                     lhsT=st_sb[c][:dn, t * 128:(t + 1) * 128],
                                     rhs=pext_sb[c][:dn, :],
                                     start=(c == 0), stop=(c == 2))
                pvs = SC.tile([128, GCOL], BF16, tag="pvs", bufs=4)
                nc.scalar.copy(out=pvs[:], in_=pvo[:])
                nc.sync.dma_start(out=pv_shard[t * 128:(t + 1) * 128, :],
                                  in_=pvs[:])

            # ---- P2: AllGather (bf16, 6.4MB out) ----
            nc.gpsimd.collective_compute(
                "AllGather", mybir.AluOpType.bypass,
                replica_groups=[list(range(NCORE))],
                ins=[pv_shard[:].opt()], outs=[gtab[:].opt()],
            )

            # ---- gathers: 4 queues concurrently per side ----
            gr_t = {}
            for side in ("u", "i"):
                for qi, (t0, t1) in enumerate(BOUNDS):
                    ntb = t1 - t0
                    g = GR.tile([128, 32 * 128], BF16, tag=f"gr_{qi}", bufs=2)
                    gr_t[(side, qi)] = g
                    g3u = g[:].bitcast(U16).rearrange("p (t e) -> p t e", e=128)
                    nc.gpsimd.dma_gather(
                        out_ap=g3u[:, 0:ntb, :], in_ap=gtab_pairs,
                        idxs_ap=idx_sb[side][:, t0 * 8:t1 * 8],
                        num_idxs=ntb * 128, num_idxs_reg=ntb * 128,
                        elem_size=128, single_packet=False, queue_num=qi)

            # ---- P4 per side ----
            reps = {}
            for side in ("u", "i"):
                # parity select: sel[p,t,:] = pair[p,t, parity*64 : +64]
                sel = BG.tile([128, 125 * GCOL], BF16, tag=f"sel_{side}")
                sel3 = sel[:].rearrange("p (t e) -> p t e", e=GCOL)
                for qi, (t0, t1) in enumerate(BOUNDS):
                    ntb = t1 - t0
                    g3 = gr_t[(side, qi)][:].rearrange("p (t e) -> p t e", e=128)
                    nc.scalar.copy(out=sel3[:, t0:t1, :],
                                   in_=g3[:, 0:ntb, 0:GCOL])
                    mask3 = par_sb[side][:, t0:t1].unsqueeze(2) \
                                                  .to_broadcast([128, ntb, GCOL])
                    nc.vector.copy_predicated(out=sel3[:, t0:t1, :], mask=mask3,
                                              data=g3[:, 0:ntb, GCOL:2 * GCOL])
                adoc = sel3[:, :, 0:50].rearrange("p t (a h) -> p t a h", a=A)
                # g0/g2 to f32 for shift adds + PE edge fixups
                g0f = WK.tile([128, 625], F32, tag="g0f")
                g0f3 = g0f[:].rearrange("p (t a) -> p t a", a=A)
                nc.vector.tensor_copy(out=g0f3, in_=sel3[:, :, 50:55])
                g2f = WK.tile([128, 625], F32, tag="g2f")
                g2f3 = g2f[:].rearrange("p (t a) -> p t a", a=A)
                nc.vector.tensor_copy(out=g2f3, in_=sel3[:, :, 55:60])

                # center logit lgc[p,t,a] = sum_h adoc * E1   (bf16 mult)
                wct = BG.tile([128, 6250], BF16, tag="w", bufs=1)
                wct4 = wct[:].rearrange("p (t a h) -> p t a h", a=A, h=H1)
                e1b = e1c_sb[:].rearrange("p (a h) -> p a h", a=A) \
                               .unsqueeze(1).to_broadcast([128, 125, A, H1])
                nc.vector.tensor_tensor(out=wct4, in0=adoc, in1=e1b, op=MUL)
                lg = WK.tile([128, 625], F32, tag="lg")     # [p, t, a]
                lg3 = lg[:].rearrange("p (t a) -> p t a", a=A)
                nc.vector.tensor_reduce(out=lg3, in_=wct4,
                                        axis=mybir.AxisListType.X,
                                        op=mybir.AluOpType.add)
                # window shifts along t
                nc.vector.tensor_tensor(out=lg3[:, 1:125, :], in0=lg3[:, 1:125, :],
                                        in1=g0f3[:, 0:124, :], op=ADD)
                nc.vector.tensor_tensor(out=lg3[:, 0:124, :], in0=lg3[:, 0:124, :],
                                        in1=g2f3[:, 1:125, :], op=ADD)
                # cross-quarter edges via PE shift matrices
                e0 = PS.tile([128, A], F32, tag="sps", bufs=2)
                nc.tensor.matmul(out=e0[:], lhsT=shdn_sb[:], rhs=g0f3[:, 124, :],
                                 start=True, stop=True)
                nc.vector.tensor_tensor(out=lg3[:, 0, :], in0=lg3[:, 0, :],
                                        in1=e0[:], op=ADD)
                e1m = PS.tile([128, A], F32, tag="sps", bufs=2)
                nc.tensor.matmul(out=e1m[:], lhsT=shup_sb[:], rhs=g2f3[:, 0, :],
                                 start=True, stop=True)
                nc.vector.tensor_tensor(out=lg3[:, 124, :], in0=lg3[:, 124, :],
                                        in1=e1m[:], op=ADD)

                # softmax over l (no max shift; logits are tiny)
                E = WK.tile([128, 625], F32, tag="E")
                nc.scalar.activation(out=E[:], in_=lg[:],
                                     func=mybir.ActivationFunctionType.Exp)
                E3 = E[:].rearrange("p (t a) -> p t a", a=A)
                Eat = E[:].rearrange("p (t a) -> p a t", a=A)
                S = SC.tile([128, A], F32, tag="S")
                nc.vector.tensor_reduce(out=S[:], in_=Eat,
                                        axis=mybir.AxisListType.X,
                                        op=mybir.AluOpType.add)
                sit = PS.tile([BLOC, A], F32, tag="sps", bufs=2)
                nc.tensor.matmul(out=sit[:], lhsT=p4sel_sb[:], rhs=S[:],
                                 start=True, stop=True)
                srec = SC.tile([BLOC, A], F32, tag="srec")
                nc.vector.reciprocal(out=srec[:], in_=sit[:])
                sbc = PS.tile([128, A], F32, tag="sps", bufs=2)
                nc.tensor.matmul(out=sbc[:], lhsT=p4selT_sb[:], rhs=srec[:],
                                 start=True, stop=True)
                attn = WK.tile([128, 625], BF16, tag="attn")
                attn3 = attn[:].rearrange("p (t a) -> p t a", a=A)
                sbc3 = sbc[:].unsqueeze(1).to_broadcast([128, 125, A])
                nc.vector.tensor_tensor(out=attn3, in0=E3, in1=sbc3, op=MUL)

                # rep: weighted sum of adoc over l, then cross-quarter sum
                wad = BG.tile([128, 6250], BF16, tag="w", bufs=1)
                wad4 = wad[:].rearrange("p (t a h) -> p t a h", a=A, h=H1)
                attnb = attn3.unsqueeze(3).to_broadcast([128, 125, A, H1])
                nc.vector.tensor_tensor(out=wad4, in0=adoc, in1=attnb, op=MUL)
                wsum = WK.tile([128, 50], F32, tag="wsum")
                wad_aht = wad[:].rearrange("p (t ah) -> p ah t", ah=50)
                nc.vector.tensor_reduce(out=wsum[:], in_=wad_aht,
                                        axis=mybir.AxisListType.X,
                                        op=mybir.AluOpType.add)
                repp = PS.tile([BLOC, 50], F32, tag="sps", bufs=2)
                nc.tensor.matmul(out=repp[:], lhsT=p4sel_sb[:], rhs=wsum[:],
                                 start=True, stop=True)
                rep = WK.tile([BLOC, 50], F32, tag=f"rep_{side}", bufs=1)
                nc.vector.tensor_copy(out=rep[:], in_=repp[:])
                reps[side] = rep

            # ---- co-attention (all [32, *] DVE ops) ----
            mexp_sb = CP.tile([BLOC, 100], F32)
            nc.sync.dma_start(out=mexp_sb[:], in_=m_exp[:])
            up_sb = CP.tile([BLOC, 500], F32)
            nc.sync.dma_start(out=up_sb[:], in_=upT_exp[:])
            ip_sb = CP.tile([BLOC, 500], F32)
            nc.sync.dma_start(out=ip_sb[:], in_=ipT_exp[:])
            uw_sb = CP.tile([BLOC, 50], F32)
            nc.sync.dma_start(out=uw_sb[:], in_=uw_exp[:])
            iw_sb = CP.tile([BLOC, 50], F32)
            nc.sync.dma_start(out=iw_sb[:], in_=iw_exp[:])

            ru, ri = reps["u"][:], reps["i"][:]
            ru3 = ru.rearrange("p (a h) -> p a h", a=A)     # [32, 5, 10]
            ri3 = ri.rearrange("p (c k) -> p c k", c=A)
            mexp3 = mexp_sb[:].rearrange("p (h k) -> p h k", h=H1)

            # UdM[b,(a,k)] = sum_h Ud[b,(a,h)] * M[h,k]
            UdM = WK.tile([BLOC, 50], F32, tag="UdM")
            UdM3 = UdM[:].rearrange("p (a k) -> p a k", a=A)
            s50 = SC.tile([BLOC, 50], F32, tag="s50")
            s50_3 = s50[:].rearrange("p (a k) -> p a k", a=A)
            for h in range(H1):
                in0 = ru3[:, :, h].unsqueeze(2).to_broadcast([BLOC, A, H1])
                in1 = mexp3[:, h, :].unsqueeze(1).to_broadcast([BLOC, A, H1])
                nc.vector.tensor_tensor(out=(UdM3 if h == 0 else s50_3),
                                        in0=in0, in1=in1, op=MUL)
                if h > 0:
                    nc.vector.tensor_tensor(out=UdM[:], in0=UdM[:], in1=s50[:], op=ADD)
            # aff[b,(a,c)] = relu(sum_k UdM[b,(a,k)] * Id[b,(c,k)])
            aff0 = WK.tile([BLOC, 25], F32, tag="aff0")
            aff0_3 = aff0[:].rearrange("p (a c) -> p a c", a=A)
            s25 = SC.tile([BLOC, 25], F32, tag="s25")
            s25_3 = s25[:].rearrange("p (a c) -> p a c", a=A)
            for k in range(H1):
                in0 = UdM3[:, :, k].unsqueeze(2).to_broadcast([BLOC, A, A])
                in1 = ri3[:, :, k].unsqueeze(1).to_broadcast([BLOC, A, A])
                nc.vector.tensor_tensor(out=(aff0_3 if k == 0 else s25_3),
                                        in0=in0, in1=in1, op=MUL)
                if k > 0:
                    nc.vector.tensor_tensor(out=aff0[:], in0=aff0[:], in1=s25[:], op=ADD)
            aff = WK.tile([BLOC, 25], F32, tag="aff")
            nc.vector.tensor_scalar_max(out=aff[:], in0=aff0[:], scalar1=0.0)
            aff3 = aff[:].rearrange("p (a c) -> p a c", a=A)

            # Hu1[b,(e,a)] = sum_h up[e,h] Ud[b,(a,h)];  Hi1 likewise
            def proj_h(dst, w_sb, r3):
                dst3 = dst[:].rearrange("p (e a) -> p e a", e=H2)
                s250 = SC.tile([BLOC, 250], F32, tag="s250")
                s250_3 = s250[:].rearrange("p (e a) -> p e a", e=H2)
                w3 = w_sb[:].rearrange("p (h e) -> p h e", h=H1)
                for h in range(H1):
                    in0 = r3[:, :, h].unsqueeze(1).to_broadcast([BLOC, H2, A])
                    in1 = w3[:, h, :].unsqueeze(2).to_broadcast([BLOC, H2, A])
                    nc.vector.tensor_tensor(out=(dst3 if h == 0 else s250_3),
                                            in0=in0, in1=in1, op=MUL)
                    if h > 0:
                        nc.vector.tensor_tensor(out=dst[:], in0=dst[:],
                                                in1=s250[:], op=ADD)

            Hu1 = WK.tile([BLOC, 250], F32, tag="Hu1")
            proj_h(Hu1, up_sb, ru3)
            Hi1 = WK.tile([BLOC, 250], F32, tag="Hi1")
            proj_h(Hi1, ip_sb, ri3)

            # Hu = relu(Hu1 + sum_c Hi1[b,(e,c)] aff[b,(a,c)])
            # Hi = relu(Hi1 + sum_a Hu1[b,(e,a)] aff[b,(a,c)])
            def coatt(dst, h1_self, h1_other, sum_over_c):
                acc = WK.tile([BLOC, 250], F32, tag=f"acc_{sum_over_c}")
                h1o3 = h1_other[:].rearrange("p (e x) -> p e x", e=H2)
                s250b = SC.tile([BLOC, 250], F32, tag="s250b")
                for c in range(A):
                    in0 = h1o3[:, :, c].unsqueeze(2).to_broadcast([BLOC, H2, A])
                    if sum_over_c:   # out index a; aff[:, a, c]
                        in1 = aff3[:, :, c].unsqueeze(1).to_broadcast([BLOC, H2, A])
                    else:            # out index c'; aff[:, c(=a), c']
                        in1 = aff3[:, c, :].unsqueeze(1).to_broadcast([BLOC, H2, A])
                    nc.vector.tensor_tensor(
                        out=s250b[:].rearrange("p (e a) -> p e a", e=H2),
                        in0=in0, in1=in1, op=MUL)
                    src = h1_self[:] if c == 0 else acc[:]
                    nc.vector.tensor_tensor(out=acc[:], in0=src, in1=s250b[:], op=ADD)
                nc.vector.tensor_scalar_max(out=dst[:], in0=acc[:], scalar1=0.0)

            Hu = WK.tile([BLOC, 250], F32, tag="Hu")
            coatt(Hu, Hu1, Hi1, sum_over_c=True)
            Hi = WK.tile([BLOC, 250], F32, tag="Hi")
            coatt(Hi, Hi1, Hu1, sum_over_c=False)

            # imp logits lu[b,a] = sum_e uw[e] Hu[b,(e,a)]
            def imp(dst5, Hx, wx_sb):
                s250c = SC.tile([BLOC, 250], F32, tag="s250c")
                nc.vector.tensor_tensor(
                    out=s250c[:].rearrange("p (e a) -> p e a", e=H2),
                    in0=Hx[:].rearrange("p (e a) -> p e a", e=H2),
                    in1=wx_sb[:].unsqueeze(2).to_broadcast([BLOC, H2, A]), op=MUL)
                v = s250c[:].rearrange("p (e a) -> p a e", e=H2)
                nc.vector.tensor_reduce(out=dst5, in_=v, axis=mybir.AxisListType.X,
                                        op=mybir.AluOpType.add)

            lu = SC.tile([BLOC, A], F32, tag="lu")
            imp(lu[:], Hu, uw_sb)
            li = SC.tile([BLOC, A], F32, tag="li")
            imp(li[:], Hi, iw_sb)
            eu = SC.tile([BLOC, A], F32, tag="eu")
            nc.scalar.activation(out=eu[:], in_=lu[:],
                                 func=mybir.ActivationFunctionType.Exp)
            ei = SC.tile([BLOC, A], F32, tag="ei")
            nc.scalar.activation(out=ei[:], in_=li[:],
                                 func=mybir.ActivationFunctionType.Exp)
            su = SC.tile([BLOC, 1], F32, tag="su")
            nc.vector.tensor_reduce(out=su[:], in_=eu[:], axis=mybir.AxisListType.X,
                                    op=mybir.AluOpType.add)
            si = SC.tile([BLOC, 1], F32, tag="si")
            nc.vector.tensor_reduce(out=si[:], in_=ei[:], axis=mybir.AxisListType.X,
                                    op=mybir.AluOpType.add)
            sur = SC.tile([BLOC, 1], F32, tag="sur")
            nc.vector.reciprocal(out=sur[:], in_=su[:])
            sir = SC.tile([BLOC, 1], F32, tag="sir")
            nc.vector.reciprocal(out=sir[:], in_=si[:])

            # ar[b,a] = sum_h Ud*Id
            arm = SC.tile([BLOC, 50], F32, tag="arm")
            nc.vector.tensor_tensor(out=arm[:], in0=ru, in1=ri, op=MUL)
            ar5 = SC.tile([BLOC, A], F32, tag="ar5")
            nc.vector.tensor_reduce(out=ar5[:],
                                    in_=arm[:].rearrange("p (a h) -> p a h", a=A),
                                    axis=mybir.AxisListType.X, op=mybir.AluOpType.add)
            # R = sum_a eu*ei*ar / (su*si) + bias
            pr = SC.tile([BLOC, A], F32, tag="pr")
            nc.vector.tensor_tensor(out=pr[:], in0=eu[:], in1=ei[:], op=MUL)
            nc.vector.tensor_tensor(out=pr[:], in0=pr[:], in1=ar5[:], op=MUL)
            r0 = SC.tile([BLOC, 1], F32, tag="r0")
            nc.vector.tensor_reduce(out=r0[:], in_=pr[:], axis=mybir.AxisListType.X,
                                    op=mybir.AluOpType.add)
            nc.vector.tensor_tensor(out=r0[:], in0=r0[:], in1=sur[:], op=MUL)
            nc.vector.tensor_tensor(out=r0[:], in0=r0[:], in1=sir[:], op=MUL)

            bias_sb = SC.tile([BLOC, 1], F32, tag="bias")
            nc.sync.dma_start(out=bias_sb[:], in_=bias[:])
            nc.vector.tensor_tensor(out=r0[:], in0=r0[:], in1=bias_sb[:], op=ADD)
            nc.sync.dma_start(out=out_ext[:], in_=r0[:])

    nc.finalize()
    return nc


_NC_CACHE = {}
_LAST_IN_MAPS = None


def _idx_layout(ids, docs):
    """idx16 [128,1000] int16 (pair idx) + parity [128,125] u8 for one side."""
    j = np.arange(NTOK)
    p = j % 128
    t = j // 128
    item = p // 4
    l = 125 * (p % 4) + t
    tok = docs[ids[item], l].astype(np.int64)          # [NTOK]
    blk = np.zeros((16, 1000), np.int16)
    blk[j % 16, j // 16] = (tok // 2).astype(np.int16)
    idx16 = np.tile(blk, (8, 1))       # replicated across the 8 Q7 cores
    par = np.zeros((128, 125), np.uint8)
    par[p, t] = (tok % 2).astype(np.uint8)
    return idx16, par


def kernel(U_ids, I_ids, U_docs, I_docs, words_emb, aspect_emb, aspect_proj,
           M, user_proj, user_w, item_proj, item_w, Bu, Bi, Bg):
    U_ids = np.asarray(U_ids).astype(np.int64).reshape(B)
    I_ids = np.asarray(I_ids).astype(np.int64).reshape(B)
    U_docs = np.asarray(U_docs).astype(np.int64)
    I_docs = np.asarray(I_docs).astype(np.int64)
    words_emb = np.asarray(words_emb, np.float32)
    aspect_emb = np.asarray(aspect_emb, np.float32)
    aspect_proj = np.asarray(aspect_proj, np.float32)
    M = np.asarray(M, np.float32)
    user_proj = np.asarray(user_proj, np.float32)
    user_w = np.asarray(user_w, np.float32)
    item_proj = np.asarray(item_proj, np.float32)
    item_w = np.asarray(item_w, np.float32)
    Bu = np.asarray(Bu, np.float32); Bi = np.asarray(Bi, np.float32)
    Bg = np.float32(np.asarray(Bg))

    # ---- host-side parameter prep ----
    pext = np.zeros((D, GCOL), np.float32)
    for a in range(A):
        pext[:, a * 10:(a + 1) * 10] = aspect_proj[a]
    for a in range(A):
        pext[:, 50 + a] = aspect_proj[a] @ aspect_emb[a, 0:10]        # g0 (w=0)
        pext[:, 55 + a] = aspect_proj[a] @ aspect_emb[a, 20:30]       # g2 (w=2)

    words_pad = np.zeros((VPAD, D), np.float32)
    words_pad[:V] = words_emb

    pr = np.arange(128)
    e1 = np.empty((128, 50), np.float32)
    for a in range(A):
        e1[:, a * 10:(a + 1) * 10] = aspect_emb[a, 10:20][None, :]
    consts = {
        "p4sel": (pr[:, None] // 4 == np.arange(BLOC)[None, :]).astype(np.float32),
        "p4selT": (pr[None, :] // 4 == np.arange(BLOC)[:, None]).astype(np.float32),
        "shdn": ((pr[None, :] == pr[:, None] + 1) &
                 (pr[None, :] % 4 != 0)).astype(np.float32),
        "shup": ((pr[None, :] == pr[:, None] - 1) &
                 (pr[None, :] % 4 != 3)).astype(np.float32),
        "e1c": e1.astype(ml_dtypes.bfloat16),
        "pext": pext.astype(ml_dtypes.bfloat16),
    }
    consts["m_exp"] = np.tile(M.reshape(1, 100), (BLOC, 1)).astype(np.float32)
    consts["upT_exp"] = np.tile(user_proj.T.reshape(1, 500), (BLOC, 1)).astype(np.float32)
    consts["ipT_exp"] = np.tile(item_proj.T.reshape(1, 500), (BLOC, 1)).astype(np.float32)
    consts["uw_exp"] = np.tile(user_w.reshape(1, 50), (BLOC, 1)).astype(np.float32)
    consts["iw_exp"] = np.tile(item_w.reshape(1, 50), (BLOC, 1)).astype(np.float32)

    in_maps = []
    for c in range(NCORE):
        uids = U_ids[c * BLOC:(c + 1) * BLOC]
        iids = I_ids[c * BLOC:(c + 1) * BLOC]
        m = dict(consts)
        m["u_idx16"], m["u_par"] = _idx_layout(uids, U_docs)
        m["i_idx16"], m["i_par"] = _idx_layout(iids, I_docs)
        m["my_shardT"] = np.ascontiguousarray(
            words_pad[c * SHARD:(c + 1) * SHARD].T).astype(ml_dtypes.bfloat16)
        m["bias"] = (Bu[uids] + Bi[iids] + Bg).astype(np.float32)[:, None].copy()
        in_maps.append(m)

    if "nc" not in _NC_CACHE:
        _NC_CACHE["nc"] = _build_nc()
    nc = _NC_CACHE["nc"]
    global _LAST_IN_MAPS
    _LAST_IN_MAPS = in_maps

    res = run_bass_kernel_spmd(nc, in_maps, core_ids=list(range(NCORE)))
    out = np.concatenate([np.asarray(res.results[c]["out"]).reshape(BLOC)
                          for c in range(NCORE)])
    return out.astype(np.float32)
